# revision 3
# baseline (speedup 1.0000x reference)
"""Dice coefficient metric kernel for TRN2 (8 NeuronCores, SPMD batch-parallel).

Reference computation (all fp32):
    inter[b,c] = sum_hw prd*tgt
    union[b,c] = sum_hw prd + sum_hw tgt + EPS
    dice[b,c]  = (2*inter + EPS) / union
    out[c]     = mean_b dice[b,c]

Sharding: batch dim (16) split across 8 cores -> 2 batches (8 (b,c) slabs
of 1024x1024) per core.  All slabs stream HBM->SBUF on the single SP
HWDGE ring as [128, 2048] quarter-slab tiles (the last slab as [128,
1024] eighths so the post-DMA drain is one eighth's compute), 10-deep
buffered.  Deep buffering matters: with 4 buffers the per-unit cadence
on HBM-contended cores was latency-bound (~12.3us/unit vs 9.8 on
uncontended cores) because the loop DMA->sem->compute->buffer-free->
issue->HBM round trip is ~50us; 10 in-flight units keep the request
queue deep enough to stay bandwidth-bound (and a deep queue also
competes better in the HBM-stack arbitration against the paired core).

Compute is split across engines so no engine comes close to the DMA
floor (both fused reductions on the DVE = 145us busy vs a ~158-190us
DMA time made every bandwidth dip a buffer-recycle stall):
  - DVE: one fused scalar_tensor_tensor per tile (prd*tgt product with
    accum -> inter partial), ~73us total.
  - ACT: two activation(Copy, accum_out) ops per tile (sum prd, sum tgt
    -> union partials), ~125us total.  ACT does no DMA issue (an ACT
    compute op in front of a DMA issue delays it and starves the ring,
    measured +31us), which is why all loads sit on the SP ring.
  - PE: two tiny ones-vector matmuls collapse the 128 partitions into
    PSUM; the per-unit partials (108 floats) are DMAd out and the final
    fold / divide / batch-mean runs on the host in fp64 during the
    gather.  (Computing dice on-device needs nc.vector.reciprocal,
    whose DVE lookup table costs a ~3us DMA preamble on every core.)

The device-side AllReduce variant was dropped: on this runtime a
16-byte 8-core AllReduce measures ~98us of fixed latency (half the
kernel's runtime), and HWDGE DMA deadlocks when a collective is present
in the NEFF, forcing slower SWDGE loads on top.  tensor_tensor_reduce
crashes the exec unit on this runtime; scalar_tensor_tensor expresses
the same fused multiply + reduction.
"""

import numpy as np

import concourse.bass as bass
import concourse.tile as tile
from concourse import bacc, mybir
from concourse.bass_utils import run_bass_kernel_spmd

B, C, H, W = 16, 4, 1024, 1024
N_CORES = 8
P = 128
EPS = 1e-6

B_LOC = B // N_CORES          # batches per core
SLABS = B_LOC * C             # (b,c) slabs per core
F = (H * W) // P              # free dim per full slab


def _build_units(slabs: int, feat: int):
    """(slab, col_offset, width) load/reduce units: full slabs in
    quarters, the last slab in eighths (shorter post-DMA drain)."""
    quarter = feat // 4
    eighth = feat // 8
    units = []
    for s in range(slabs - 1):
        for q in range(4):
            units.append((s, q * quarter, quarter))
    for e in range(8):
        units.append((slabs - 1, e * eighth, eighth))
    return units


def _build_nc(slabs: int, feat: int, c: int, n_cores: int):
    """Build + compile the per-core Bass program (same program on all cores)."""
    nc = bacc.Bacc(
        "TRN2", target_bir_lowering=False, debug=False, num_devices=n_cores
    )
    f32 = mybir.dt.float32
    units = _build_units(slabs, feat)
    n_units = len(units)
    quarter = feat // 4
    prd = nc.dram_tensor("prd", [slabs, P, feat], f32, kind="ExternalInput")
    tgt = nc.dram_tensor("tgt", [slabs, P, feat], f32, kind="ExternalInput")
    out = nc.dram_tensor("out", [1, 3 * n_units], f32, kind="ExternalOutput")

    copy_fn = mybir.ActivationFunctionType.Copy
    add = mybir.AluOpType.add
    mult = mybir.AluOpType.mult

    with tile.TileContext(nc) as tc:
        with (
            tc.tile_pool(name="io", bufs=10) as io_pool,
            tc.tile_pool(name="work", bufs=1) as work_pool,
            tc.tile_pool(name="psum", bufs=1, space=bass.MemorySpace.PSUM) as psum_pool,
        ):
            # Per-partition partials.  DVE and ACT write separate stats
            # tiles (sharing one would cross-serialize their queues);
            # each collapses with its own ones-vector matmul.
            # stats_int col u = inter partial of unit u; stats_sum col u
            # = prd-sum, col n_units+u = tgt-sum of unit u.
            stats_int = work_pool.tile([P, n_units], f32)
            stats_sum = work_pool.tile([P, 2 * n_units], f32)
            dve_scr = work_pool.tile([P, quarter], f32)
            act_scr = work_pool.tile([P, quarter], f32)

            for u, (s, off, width) in enumerate(units):
                pt = io_pool.tile([P, width], f32, tag="prd")
                nc.sync.dma_start(pt[:], prd[s, :, off : off + width])
                tt = io_pool.tile([P, width], f32, tag="tgt")
                nc.sync.dma_start(tt[:], tgt[s, :, off : off + width])

                # DVE: inter partial = sum((pt * 1) * tt)
                nc.vector.scalar_tensor_tensor(
                    out=dve_scr[:, 0:width], in0=pt[:], scalar=1.0, in1=tt[:],
                    op0=mult, op1=mult,
                    accum_out=stats_int[:, u : u + 1],
                )
                # ACT: union partials = sum(pt), sum(tt)
                nc.scalar.activation(
                    out=act_scr[:, 0:width], in_=pt[:], func=copy_fn,
                    accum_out=stats_sum[:, u : u + 1],
                )
                nc.scalar.activation(
                    out=act_scr[:, 0:width], in_=tt[:], func=copy_fn,
                    accum_out=stats_sum[:, n_units + u : n_units + u + 1],
                )

            # Collapse the 128 partitions: ps[0, :] = ones.T @ stats (PSUM),
            # bounce through SBUF (DMA has no PSUM route), DMA the 3*n_units
            # partials out.
            ones = work_pool.tile([P, 1], f32)
            nc.vector.memset(ones[:], 1.0)
            ps_int = psum_pool.tile([1, n_units], f32)
            nc.tensor.matmul(ps_int[:], ones[:], stats_int[:], start=True, stop=True)
            ps_sum = psum_pool.tile([1, 2 * n_units], f32)
            nc.tensor.matmul(ps_sum[:], ones[:], stats_sum[:], start=True, stop=True)

            fin = work_pool.tile([1, 3 * n_units], f32)
            nc.vector.tensor_copy(fin[0:1, 0:n_units], ps_int[:])
            nc.vector.tensor_copy(fin[0:1, n_units : 3 * n_units], ps_sum[:])
            nc.sync.dma_start(out[0:1, :], fin[:])

    nc.compile()
    return nc


_NC_CACHE: dict = {}


def _get_nc():
    key = (SLABS, F, C, N_CORES)
    if key not in _NC_CACHE:
        _NC_CACHE[key] = _build_nc(*key)
    return _NC_CACHE[key]


def _shard_inputs(prd: np.ndarray, tgt: np.ndarray):
    in_maps = []
    for i in range(N_CORES):
        sl = slice(i * B_LOC, (i + 1) * B_LOC)
        in_maps.append(
            {
                "prd": np.ascontiguousarray(prd[sl]).reshape(SLABS, P, F),
                "tgt": np.ascontiguousarray(tgt[sl]).reshape(SLABS, P, F),
            }
        )
    return in_maps


def _gather(core_outs, slabs: int, feat: int, c: int) -> np.ndarray:
    """Fold per-unit partials from all cores into the final per-channel
    dice mean (fp64 on host)."""
    units = _build_units(slabs, feat)
    n_units = len(units)
    slab_of_unit = np.array([s for s, _, _ in units])
    dice_sum = np.zeros(c, dtype=np.float64)
    n_b = 0
    for raw in core_outs:
        v = np.asarray(raw, dtype=np.float64).reshape(3 * n_units)
        ints, psums, tsums = v[:n_units], v[n_units : 2 * n_units], v[2 * n_units :]
        inter = np.zeros(slabs)
        usum = np.zeros(slabs)
        np.add.at(inter, slab_of_unit, ints)
        np.add.at(usum, slab_of_unit, psums + tsums)
        dice = (2.0 * inter + EPS) / (usum + EPS)          # per (b_loc, c) slab
        dice_sum += dice.reshape(-1, c).sum(axis=0)
        n_b += slabs // c
    return (dice_sum / n_b).astype(np.float32)


def kernel(prd: np.ndarray, tgt: np.ndarray, _trace: bool = False):
    prd = np.asarray(prd, dtype=np.float32)
    tgt = np.asarray(tgt, dtype=np.float32)
    assert prd.shape == (B, C, H, W) and tgt.shape == (B, C, H, W)

    nc = _get_nc()
    in_maps = _shard_inputs(prd, tgt)
    res = run_bass_kernel_spmd(nc, in_maps, list(range(N_CORES)), trace=_trace)
    out = _gather([r["out"] for r in res.results], SLABS, F, C)
    if _trace:
        return out, res
    return out
